# revision 20
# baseline (speedup 1.0000x reference)
"""Trainium2 Bass kernel for nn_DotProductAttention_6030134084023.

reference: softmax(mask(Q @ K^T / sqrt(64), valid_lens)) @ V
  query/key/value: [64, 1024, 64] f32, valid_lens: [64] int32 -> [64, 1024, 64] f32

Strategy (v2)
-------------
Batch dim sharded across the 8 NeuronCores; host sorts batches by valid_len
(descending) and deals them round-robin so slot s holds similar lengths on
every core. The kernel is compiled per valid_lens pattern (chunk counts are
specialized; correctness never depends on the specialization since skipped
chunks are exactly-masked).

All matmuls in bf16 (fp32 PSUM accumulation); host pre-casts/transposes
Q,K,V (layout+dtype only) and post-casts the output.

Per-core dataflow per batch slot, in "S^T orientation" (keys on SBUF
partitions, queries on the free dim — no transposes anywhere):

  ST_c[k, q]  = KT_c.T @ QT        PE bf16; chunk pairs run CONCURRENTLY
                                   via tile_position row packing (K=64)
  EST_c       = exp(0.125 ST + m)  masking folded into the exp:
                                   - ScalarE path: per-partition bias 0/-80
                                   - DVE path: custom 2-op exp (deg-2 poly on
                                     t=s/2048, 8 squarings) with 0/1
                                     per-partition mask multiply
  UT[:, q]   += Vm_c.T @ EST_c     PE bf16 K=128, PSUM-accumulated, where
                                   Vm_c = [V_c | ones*64] ([128, 128]) so UT
                                   rows 64..127 hold the softmax denominator
                                   replicated 64x (makes normalize lane-local)

The exp work is split between ScalarE (the 1 elem/lane/cycle bottleneck) and
the custom DVE exp to balance engine load. Postprocess per (slot, half):
DMA UT PSUM->SBUF (no engine time), DVE reciprocal_approx_fast on the
replicated denominator rows, GPSIMD elementwise multiply -> bf16 out DMA.
"""

import re

import numpy as np
import ml_dtypes

import concourse.bass as bass
import concourse.bacc as bacc
import concourse.tile as tile
from concourse import mybir
from concourse import bass_utils
from concourse import dve_ops as _dve_ops
from concourse.dve_ops import DveOp
from concourse.dve_spec import Spec, Src0, Src1, One, C0, C1, C2, sq

F32 = mybir.dt.float32
BF16 = mybir.dt.bfloat16
I32 = mybir.dt.int32
AF = mybir.ActivationFunctionType
ALU = mybir.AluOpType

NCORES = 8
B = 64
S = 1024
D = 64
BPC = B // NCORES  # 8 batch slots per core
KC = S // 128  # 8 k-chunks of 128
QH = 512  # q-half

NEG_BIAS = -80.0  # exp(0.125*s - 80) ~ 0 for any |s| <= 50
ALPHA = 0.125 / 256.0  # exp arg prescale for the DVE poly path (8 squarings)
DVE_EXP_MOD = 5  # every DVE_EXP_MOD-th chunk's exp runs on DVE (0 = ACT only)

_BUILD_CACHE = {}


# ---------------------------------------------------------------------------
# Custom DVE exp: est = ((b2*s + b1)*s + 1)^256 * mask  ~=  exp(0.125*s)*mask
# op1: p = ((C1*s + C2)*s + One) * C0   (C0 = per-partition 0/1 mask)
# op2: p^256 via 8 squarings
# ---------------------------------------------------------------------------

def _register_op(name, spec):
    """Register a new custom-DVE op at runtime: reserve a free opcode row,
    then compile once to learn (and pin) the uops sha."""
    from concourse.dve_table_gen import free_opcode_rows

    if name in _dve_ops._SUB_OPCODE_FOR_NAME:
        return next(o for o in _dve_ops.OPS if o.name == name)
    row = _dve_ops._CUSTOM_DVE_ROW_BASE + len(_dve_ops.OPS)
    if row >= 0x20 or row not in set(free_opcode_rows("TRN2")):
        raise RuntimeError(f"no free custom-DVE row for {name}")
    op = DveOp(name, spec, subdim=False, uops_sha={})
    _dve_ops.OPS.append(op)
    _dve_ops._SUB_OPCODE_FOR_NAME[name] = row
    try:
        op.compile("v3")
    except ValueError as e:
        m = re.search(r"v3: ([0-9a-f]+)", str(e))
        if not m:
            _dve_ops.OPS.remove(op)
            del _dve_ops._SUB_OPCODE_FOR_NAME[name]
            raise
        op = DveOp(name, spec, subdim=False, uops_sha={"v3": m.group(1)})
        _dve_ops.OPS[-1] = op
    op.compile("v3")
    _dve_ops.CUSTOM_DVE_SPECS[name] = op.spec
    return op


def _bitnot(x):
    from concourse.dve_spec import Bin, AluOp

    return Bin(AluOp.BITWISE_NOT, x, x)


def _np_bitnot(x):
    return (~np.ascontiguousarray(x, np.float32).view(np.int32)).view(np.float32)


# seed constants shared with RECIPROCAL_APPROX_FAST (1-NR variant: ~0.17% err)
_RC0, _RC1 = -0.23549792, 2.0017324


def _register_dve_ops():
    try:
        # est = ((b2*s + b1)*s + 1) * mask   (deg-2 poly of exp(s/2048))
        op1 = _register_op(
            "EXP_P2M_ANT",
            Spec(
                body=((C1 * Src0 + C2) * Src0 + One) * C0,
                reference=lambda in0, in1, s0, s1, imm2: (
                    (s1 * in0 + imm2) * in0 + 1.0
                )
                * s0,
            ),
        )
        sq8 = Src0
        for _ in range(8):
            sq8 = sq(sq8)
        op2 = _register_op(
            "EXP_SQ8_ANT",
            Spec(body=sq8, reference=lambda in0, in1, s0, s1, imm2: in0 ** 256),
        )
        # out = in1 / in0 (approx): bit-trick seed + one Newton pass, fused mult
        _y0 = _bitnot(Src0) * C0
        op3 = _register_op(
            "NRECIP_MUL_ANT",
            Spec(
                body=(_y0 * (C1 - Src0 * _y0)) * Src1,
                reference=lambda in0, in1, s0, s1, imm2: (
                    (_np_bitnot(in0) * s0)
                    * (s1 - in0 * (_np_bitnot(in0) * s0))
                )
                * in1,
            ),
        )
        return op1, op2, op3
    except Exception:
        return None


_DVE_OPS = _register_dve_ops()
_HAVE_DVE_EXP = _DVE_OPS is not None
if _DVE_OPS:
    _EXP_P2M, _EXP_SQ8, _NRECIP_MUL = _DVE_OPS
_B1 = ALPHA
_B2 = ALPHA * ALPHA / 2.0


def _build(nprocs, nreals, debug_dump=False, ncores=NCORES):
    """nreals[s]: number of 128-key chunks with any valid key for slot s."""
    nc = bacc.Bacc("TRN2", target_bir_lowering=False, debug=False, num_devices=ncores)
    qt = nc.dram_tensor("qt", [BPC, D, S], BF16, kind="ExternalInput").ap()
    kt = nc.dram_tensor("kt", [BPC, D, S], BF16, kind="ExternalInput").ap()
    # v is [V | ones]: host appends 64 ones columns so UT rows 64..127
    # accumulate the softmax denominator (replicated for lane-local normalize)
    v = nc.dram_tensor("v", [BPC, S, 128], BF16, kind="ExternalInput").ap()
    bias_t = nc.dram_tensor("bias_t", [128, KC * BPC], F32, kind="ExternalInput").ap()
    mask_t = nc.dram_tensor("mask_t", [128, KC * BPC], F32, kind="ExternalInput").ap()
    ot = nc.dram_tensor("ot", [BPC, D, S], BF16, kind="ExternalOutput").ap()
    usb_o = None
    if debug_dump:
        usb_o = nc.dram_tensor(
            "usb_o", [BPC, 128, S], F32, kind="ExternalOutput"
        ).ap()

    use_dve = _HAVE_DVE_EXP and DVE_EXP_MOD > 0

    with tile.TileContext(nc) as tc:
        with (
            tc.tile_pool(name="const", bufs=1) as constp,
            tc.tile_pool(name="pt", bufs=2) as ptp,
            tc.tile_pool(name="est", bufs=6) as estp,
            tc.tile_pool(name="usb", bufs=4) as usbp,
            tc.tile_pool(name="osb", bufs=2) as osbp,
            tc.tile_pool(name="stp", bufs=3, space="PSUM") as stp,
            tc.tile_pool(name="utp", bufs=1, space="PSUM") as utp,
        ):
            # ---- persistent SBUF inputs ----
            qt2 = constp.tile([128, BPC * S], BF16, tag="qt2")
            kt2 = constp.tile([128, BPC * S], BF16, tag="kt2")
            # vma[p, (b*KC + kc)*128 + j]: j<64 -> V[b, kc*128+p, j]; j>=64 -> 1.0
            vma = constp.tile([128, BPC * KC * 128], BF16, tag="vma")
            bias_sb = constp.tile([128, KC * BPC], F32, tag="bias")
            mask_sb = constp.tile([128, KC * BPC], F32, tag="mask")

            nc.gpsimd.dma_start(out=bias_sb[:], in_=bias_t)
            nc.gpsimd.dma_start(out=mask_sb[:], in_=mask_t)

            vview = vma[:].rearrange("p (b j) -> p b j", j=128)

            # Q/K on sync queue, V on gpsimd queue, both in slot_order-first
            # groups so slot 6's inputs land before the rest
            slot_order = [6, 0, 4, 1, 5, 2, 3, 7]
            dma_groups = [slot_order[0:1]] + [
                slot_order[1 + 2 * g : 3 + 2 * g] for g in range(4)
            ]
            for grp in dma_groups:
                for b in grp:
                    for half in (slice(0, 64), slice(64, 128)):
                        nc.sync.dma_start(
                            out=qt2[half, b * S : (b + 1) * S], in_=qt[b]
                        )
                        kwb = nreals[b] * 128
                        nc.sync.dma_start(
                            out=kt2[half, b * S : b * S + kwb], in_=kt[b, :, 0:kwb]
                        )
                    nc.gpsimd.dma_start(
                        out=vview[:, b * KC : b * KC + nreals[b], :],
                        in_=bass.AP(
                            tensor=v.tensor,
                            offset=v[b].offset,
                            ap=[[128, 128], [128 * 128, nreals[b]], [1, 128]],
                        ),
                    )

            chunk_ctr = 0

            def exp_chunk(est_t, st_t, b, kc):
                nonlocal chunk_ctr
                col = kc * BPC + b
                on_dve = use_dve and (chunk_ctr % DVE_EXP_MOD == DVE_EXP_MOD - 1)
                chunk_ctr += 1
                if on_dve:
                    p4 = ptp.tile([128, 2 * QH], F32, tag="p4")
                    nc.vector._custom_dve(
                        _EXP_P2M,
                        out=p4[:],
                        in0=st_t[:],
                        s0=mask_sb[:, col : col + 1],
                        s1=_B2,
                        imm2=_B1,
                    )
                    nc.vector._custom_dve(_EXP_SQ8, out=est_t[:], in0=p4[:])
                else:
                    nc.scalar.activation(
                        out=est_t[:],
                        in_=st_t[:],
                        func=AF.Exp,
                        scale=0.125,
                        bias=bias_sb[:, col : col + 1],
                    )

            for b in slot_order:
                nreal = nreals[b]
                # single [128, 2*QH] accumulator: h=0 in cols 0:QH, h=1 in
                # cols QH:2QH; rows 64..127 hold the replicated denominator
                ut = utp.tile([128, 2 * QH], F32, tag="ut")

                qb = qt2[:, b * S : (b + 1) * S]
                kb = kt2[:, b * S : (b + 1) * S]

                # pipeline: STs run ahead; exp per chunk; UTs deferred one
                # pair so an est-waiting UT doesn't head-of-line-block STs
                pend_ut = []  # chunks exp'd, UT not yet emitted
                pend_exp = []  # (est_t, st_t, kc) ST done, exp not yet emitted

                def emit_uts(chunks, nreal=nreal, b=b, ut=ut):
                    for kc, est_t in chunks:
                        voff = (b * KC + kc) * 128
                        for h in (0, 1):
                            nc.tensor.matmul(
                                ut[:, h * QH : (h + 1) * QH],
                                vma[:, voff : voff + 128],
                                est_t[:, h * QH : (h + 1) * QH],
                                start=(kc == 0),
                                stop=(kc == nreal - 1),
                            )

                npairs = (nreal + 1) // 2
                for p in range(npairs):
                    A, Bc = 2 * p, 2 * p + 1
                    chunks = [A] + ([Bc] if Bc < nreal else [])
                    sts = {}
                    for i, c in enumerate(chunks):
                        sts[c] = stp.tile([128, 2 * QH], F32, tag="st", name="st")
                    # ST matmuls: pair concurrent via row packing
                    for h in (0, 1):
                        hs = slice(h * QH, (h + 1) * QH)
                        for i, c in enumerate(chunks):
                            rows = slice(64 * i, 64 * i + 64)
                            nc.tensor.matmul(
                                sts[c][:, hs],
                                kb[rows, c * 128 : (c + 1) * 128],
                                qb[rows, hs],
                                start=True,
                                stop=True,
                                tile_position=(64 * i, 0),
                            )
                    # emit deferred UTs (previous pair's chunks)
                    if pend_ut:
                        emit_uts(pend_ut)
                        pend_ut = []
                    # exp this pair's chunks
                    for c in chunks:
                        est_t = estp.tile([128, 2 * QH], BF16, tag="est")
                        exp_chunk(est_t, sts[c], b, c)
                        pend_ut.append((c, est_t))
                emit_uts(pend_ut)

                # ---- postprocess: normalize by the replicated denominator ----
                usb = usbp.tile([128, 2 * QH], F32, tag="usb")
                nc.vector.tensor_copy(out=usb[:], in_=ut[:])
                if usb_o is not None:
                    nc.sync.dma_start(out=usb_o[b], in_=usb[:])
                # den lives on partitions 64..127; DVE lanes are partition-
                # local, so DMA it across to 0..63 before recip+mult
                den_lo = usbp.tile([64, 2 * QH], F32, tag="den_lo")
                nc.sync.dma_start(out=den_lo[:], in_=usb[64:128, :])
                osb = osbp.tile([64, 2 * QH], BF16, tag="osb")
                if _HAVE_DVE_EXP:
                    # fused osb = num * ~(1/den) (bit-trick seed + 1 Newton)
                    nc.vector._custom_dve(
                        _NRECIP_MUL,
                        out=osb[:],
                        in0=den_lo[:],
                        in1=usb[0:64, :],
                        s0=_RC0,
                        s1=_RC1,
                    )
                else:
                    rec = usbp.tile([64, 2 * QH], F32, tag="rec")
                    nc.vector.reciprocal_approx_fast(rec[:], den_lo[:])
                    nc.gpsimd.tensor_tensor(
                        out=osb[:], in0=usb[0:64, :], in1=rec[:], op=ALU.mult
                    )
                nc.gpsimd.dma_start(out=ot[b], in_=osb[:])

    nc.compile()
    return nc


def _plan(valid_lens):
    """Sort batches by length, deal to (slot, core); per-slot chunk counts."""
    order = np.argsort(-valid_lens, kind="stable")  # [B]
    nprocs, nreals = [], []
    for s in range(BPC):
        slot_max = int(valid_lens[order[s * NCORES]])
        nchunks = max(1, -(-slot_max // 128))  # ceil, >= 1
        nprocs.append(nchunks)
        nreals.append(nchunks)
    return order, tuple(nprocs), tuple(nreals)


plan = _plan
build = _build


def make_in_maps(query, key, value, valid_lens, order):
    bf = ml_dtypes.bfloat16
    qt = query.transpose(0, 2, 1)
    kt = key.transpose(0, 2, 1)
    iota = np.arange(128)
    in_maps = []
    for c in range(NCORES):
        idx = [int(order[s * NCORES + c]) for s in range(BPC)]
        vls = valid_lens[idx]  # [BPC]
        # mask[p, kc*BPC + b] = 1.0 if kc*128 + p < vl[b] else 0.0
        kk = (128 * np.arange(KC)[:, None, None] + iota[None, None, :])  # [KC,1,128]
        m = (kk < vls[None, :, None]).astype(np.float32)  # [KC, BPC, 128]
        mask_t = np.ascontiguousarray(
            m.transpose(2, 0, 1).reshape(128, KC * BPC)
        )
        bias_t = (mask_t - 1.0) * (-NEG_BIAS)  # 0 valid, -80 invalid
        vexts = np.concatenate(
            [value[idx], np.ones((BPC, S, 64), np.float32)], axis=2
        )
        in_maps.append(
            {
                "qt": np.ascontiguousarray(qt[idx]).astype(bf),
                "kt": np.ascontiguousarray(kt[idx]).astype(bf),
                "v": np.ascontiguousarray(vexts).astype(bf),
                "bias_t": np.ascontiguousarray(bias_t.astype(np.float32)),
                "mask_t": mask_t,
            }
        )
    return in_maps


def gather_output(results, order):
    out = np.empty((B, S, D), dtype=np.float32)
    for c in range(NCORES):
        otc = np.asarray(results[c]["ot"]).astype(np.float32)  # [BPC, D, S]
        for s in range(BPC):
            out[int(order[s * NCORES + c])] = otc[s].T
    return out


def kernel(query, key, value, valid_lens):
    query = np.ascontiguousarray(np.asarray(query, dtype=np.float32))
    key = np.ascontiguousarray(np.asarray(key, dtype=np.float32))
    value = np.ascontiguousarray(np.asarray(value, dtype=np.float32))
    valid_lens = np.asarray(valid_lens).astype(np.int32).reshape(B)
    assert query.shape == (B, S, D) and key.shape == (B, S, D)
    assert value.shape == (B, S, D)

    order, nprocs, nreals = _plan(valid_lens)
    cache_key = (nprocs, nreals)
    nc = _BUILD_CACHE.get(cache_key)
    if nc is None:
        nc = _build(nprocs, nreals)
        _BUILD_CACHE[cache_key] = nc

    in_maps = make_in_maps(query, key, value, valid_lens, order)
    res = bass_utils.run_bass_kernel_spmd(nc, in_maps, core_ids=list(range(NCORES)))
    return gather_output(res.results, order)
